# revision 20
# baseline (speedup 1.0000x reference)
"""Trainium2 Bass kernel for batched attention with LayerNorm'd projections.

Reference computation (per batch element b):
    keys    = LN(Y[b] @ K) * g1 + b1          [S, H]
    queries = LN(X[b] @ Q) * g2 + b2          [S, H]
    alpha   = softmax(queries @ keys.T / H)   [S, S]
    out[b]  = alpha @ Y[b]                    [S, F]

Shapes: B=8, S=2048, F=H=1024. Data-parallel: one batch element per
NeuronCore, 8 cores, no collectives.

Device pipeline per core:
  A: q/k projections in natural [S,H] layout (host supplies X^T, Y^T in
     fp8-e4m3 so the full operand set stays SBUF-resident; the contraction
     dim F lands on partitions), f32 PSUM accumulate, LayerNorm along the
     free dim, then PE-transpose 128x128 bf16 blocks into H-major
     queriesT/keysT.
  B: logits stripes [128, 2048] = queriesT_block^T @ keysT (bf16),
     exp(x/1024) fused on the scalar engine with row-sum accumulation
     (softmax denominator comes out of accum_out for free), PE-transpose
     the alpha stripe into Sk-major layout.
  C: U = alphaT^T @ Y (bf16) in natural [S,F] layout; the mandatory
     PSUM->SBUF copy applies the per-row 1/denominator, so no separate
     softmax normalization pass exists anywhere.

Numerics: fp8 projection inputs cost ~4e-3 relative error end-to-end
(validated against the f32 reference offline); logits/softmax/value paths
stay bf16 with f32 accumulation.

DMA instructions can carry at most 2 semaphore waits on this target (and
the XBAR transpose form only 1), so mid-pipeline transposes run on the
TensorEngine (identity matmul -> PSUM -> DVE copy), and all DMA-written
SBUF tiles are written exactly once (full fp8 residency removes streaming
slot reuse in phase A).
"""

import numpy as np
import ml_dtypes

import concourse.bass as bass
import concourse.bacc as bacc
import concourse.tile as tile
from concourse import mybir
from concourse.bass_utils import run_bass_kernel_spmd
from concourse.masks import make_identity

BF16 = mybir.dt.bfloat16
FP8 = mybir.dt.float8e4
F32 = mybir.dt.float32
AF = mybir.ActivationFunctionType

S = 2048  # sequence length per core
F = 1024  # input feature dim
H = 1024  # hidden dim
P = 128  # partitions
NS = S // P  # 16 sequence stripes
NF = F // P  # 8 contraction tiles for projections
NH = H // P  # 8 hidden tiles
NC = 512  # matmul free-dim chunk (one PSUM bank)
EPS = 1e-5


def _build_nc(affine1: bool, affine2: bool) -> bass.Bass:
    nc = bacc.Bacc(None)

    xt = nc.declare_dram_parameter("XT", [F, S], FP8, isOutput=False)[:]
    yt = nc.declare_dram_parameter("YT", [F, S], FP8, isOutput=False)[:]
    y8 = nc.declare_dram_parameter("Y8", [S, F], FP8, isOutput=False)[:]
    cs = nc.declare_dram_parameter("CS", [1, F], mybir.dt.float32r, isOutput=False)[:]
    onesp = nc.declare_dram_parameter("ONES", [1, P], mybir.dt.float32r, isOutput=False)[:]
    kw = nc.declare_dram_parameter("Kw", [F, H], FP8, isOutput=False)[:]
    qw = nc.declare_dram_parameter("Qw", [F, H], FP8, isOutput=False)[:]
    g1 = b1 = g2 = b2 = None
    if affine1:
        g1 = nc.declare_dram_parameter("g1r", [1, H], BF16, isOutput=False)[:]
        b1 = nc.declare_dram_parameter("b1r", [1, H], BF16, isOutput=False)[:]
    if affine2:
        g2 = nc.declare_dram_parameter("g2r", [1, H], BF16, isOutput=False)[:]
        b2 = nc.declare_dram_parameter("b2r", [1, H], BF16, isOutput=False)[:]
    out = nc.declare_dram_parameter("out", [S, F], F32, isOutput=True)[:]

    with tile.TileContext(nc) as tc:
        with (
            tc.tile_pool(name="persist", bufs=1) as persist,
            tc.tile_pool(name="stats", bufs=8) as stats_pool,
        ):
            # Persistent SBUF tensors (whole-kernel lifetime).
            # Per-partition: qT 32k + kT 32k + y_sb 32k + recips ~0.1k = 96k.
            qT = persist.tile([P, NH, S], FP8, tag="qT")  # queriesT [H, S]
            kT = persist.tile([P, NH, S], FP8, tag="kT")  # keysT    [H, S]
            recips = persist.tile([P, NS], F32, tag="recips")
            y_sb = persist.tile([P, NS, F], FP8, tag="y_sb")  # Y [Sk, F]
            crow = persist.tile([1, F], mybir.dt.float32r, tag="crow")  # colsum(Y)
            ones1 = persist.tile([1, P], mybir.dt.float32r, tag="ones1")
            eps_sb = persist.tile([P, 1], F32, tag="eps")
            nc.vector.memset(eps_sb, EPS)
            identb = persist.tile([P, P], BF16, tag="identb")
            make_identity(nc, identb)

            def layer_norm_apply(psums, raw, dst, gamma, beta):
                """dst = LN(concat(psums)) (*gamma+beta), free dim H.

                The ACT copies into `raw` are the only PSUM readers, so the
                banks recycle ~1us after the matmuls; the long stats chain
                (bn_stats -> aggr -> sqrt -> recip -> apply) runs off `raw`
                and only gates this stripe's transposes, not the next
                stripe's matmuls.
                """
                for i, ps in enumerate(psums):
                    nc.scalar.activation(
                        out=raw[:, i * NC : (i + 1) * NC], in_=ps, func=AF.Copy
                    )
                # bn_stats free-dim limit is 512.
                st = stats_pool.tile([P, len(psums), 6], F32, tag="bn")
                for i in range(len(psums)):
                    nc.vector.bn_stats(
                        out=st[:, i, :], in_=raw[:, i * NC : (i + 1) * NC]
                    )
                mv = stats_pool.tile([P, 2], F32, tag="mv")
                nc.vector.bn_aggr(out=mv, in_=st)
                rstd = stats_pool.tile([P, 1], F32, tag="rstd")
                nc.scalar.activation(
                    out=rstd, in_=mv[:, 1:2], func=AF.Sqrt, bias=eps_sb
                )
                nc.vector.reciprocal(out=rstd, in_=rstd)
                nbias = stats_pool.tile([P, 1], F32, tag="nbias")
                nc.vector.tensor_scalar(
                    out=nbias,
                    in0=mv[:, 0:1],
                    scalar1=rstd,
                    scalar2=-1.0,
                    op0=mybir.AluOpType.mult,
                    op1=mybir.AluOpType.mult,
                )
                nc.scalar.activation(
                    out=dst, in_=raw, func=AF.Identity, bias=nbias, scale=rstd
                )
                if gamma is not None:
                    nc.vector.tensor_mul(dst, dst, gamma)
                if beta is not None:
                    nc.vector.tensor_add(dst, dst, beta)

            # ---- Phase A: projections + LN + transpose to H-major ----
            with (
                tc.tile_pool(name="operands", bufs=1) as operands,
                tc.tile_pool(name="work", bufs=3) as work,
                tc.tile_pool(name="psumA", bufs=1, space="PSUM") as psumA,
                tc.tile_pool(name="psumAT", bufs=2, space="PSUM") as psumAT,
            ):
                # All projection operands SBUF-resident in fp8:
                # xt/yt 16k + q/k 8k each = 48k per partition.
                xt_sb = operands.tile([P, NF, S], FP8, tag="xt_sb")
                yt_sb = operands.tile([P, NF, S], FP8, tag="yt_sb")
                q_sb = operands.tile([P, NF, H], FP8, tag="q_sb")
                k_sb = operands.tile([P, NF, H], FP8, tag="k_sb")
                xt_r = xt.rearrange("(fb p) s -> p fb s", p=P)
                yt_r = yt.rearrange("(fb p) s -> p fb s", p=P)
                qw_r = qw.rearrange("(fb p) h -> p fb h", p=P)
                kw_r = kw.rearrange("(fb p) h -> p fb h", p=P)
                # Per-f-block loads so the first matmuls start after ~400KB,
                # not after the full operand set; k/yt first (k stripes run
                # first below).
                for f in range(NF):
                    nc.sync.dma_start(out=yt_sb[:, f, :], in_=yt_r[:, f, :])
                    nc.sync.dma_start(out=k_sb[:, f, :], in_=kw_r[:, f, :])
                for f in range(NF):
                    nc.sync.dma_start(out=xt_sb[:, f, :], in_=xt_r[:, f, :])
                    nc.sync.dma_start(out=q_sb[:, f, :], in_=qw_r[:, f, :])
                aff_tiles = {}
                for name, flag, ap in (
                    ("g1", affine1, g1),
                    ("b1", affine1, b1),
                    ("g2", affine2, g2),
                    ("b2", affine2, b2),
                ):
                    if flag:
                        t = operands.tile([P, H], BF16, tag=name, name=f"aff_{name}")
                        rep = bass.AP(
                            tensor=ap.tensor, offset=ap.offset, ap=[[0, P], ap.ap[1]]
                        )
                        nc.sync.dma_start(out=t, in_=rep)
                        aff_tiles[name] = t

                DR = mybir.MatmulPerfMode.DoubleRow
                mats = [("k", s) for s in range(NS)] + [("q", s) for s in range(NS)]
                for mi, (which, s) in enumerate(mats):
                    sblk = bass.ts(s, P)
                    lhs_all = xt_sb if which == "q" else yt_sb
                    rhs_all = q_sb if which == "q" else k_sb
                    # 3 rotating PSUM bank sets so the LayerNorm stats chain
                    # of stripe i drains while stripes i+1, i+2 accumulate.
                    pset = mi % 3
                    ps = [
                        psumA.tile(
                            [P, NC], F32, tag=f"p{pset}{c}", name=f"ps_{mi}_{c}"
                        )
                        for c in range(H // NC)
                    ]
                    for i in range(NF // 2):
                        for c in range(H // NC):
                            nc.tensor.matmul(
                                ps[c],
                                lhs_all[:, 2 * i : 2 * i + 2, sblk],
                                rhs_all[:, 2 * i : 2 * i + 2, c * NC : (c + 1) * NC],
                                perf_mode=DR,
                                start=(i == 0),
                                stop=(i == NF // 2 - 1),
                            )
                    raw = work.tile([P, H], BF16, tag=f"{which}_raw")
                    nat = work.tile([P, H], BF16, tag=f"{which}_nat")
                    if which == "q":
                        layer_norm_apply(
                            ps, raw, nat, aff_tiles.get("g2"), aff_tiles.get("b2")
                        )
                    else:
                        layer_norm_apply(
                            ps, raw, nat, aff_tiles.get("g1"), aff_tiles.get("b1")
                        )
                    dstT = qT if which == "q" else kT
                    for g in range(NH // 4):
                        tp = psumAT.tile(
                            [P, 4, P], BF16, tag="tpA", name=f"tp_{which}{g}"
                        )
                        for j in range(4):
                            nc.tensor.transpose(
                                tp[:, j, :],
                                nat[:, (4 * g + j) * P : (4 * g + j + 1) * P],
                                identb,
                            )
                        nc.vector.tensor_copy(
                            dstT[:, 4 * g : 4 * g + 4, sblk], tp
                        )
                # Y values for phase C: issued after the phase-A loads in
                # trace order so they don't delay the first matmuls.
                nc.sync.dma_start(
                    out=y_sb, in_=y8.rearrange("(sb p) f -> p sb f", p=P)
                )
                nc.sync.dma_start(out=crow, in_=cs)
                nc.sync.dma_start(out=ones1, in_=onesp)

            # ---- Phases B and C (interleaved per stripe) ----
            with (
                tc.tile_pool(name="workBC", bufs=3) as workBC,
                tc.tile_pool(name="psumB", bufs=1, space="PSUM") as psumB,
                tc.tile_pool(name="psumBT", bufs=2, space="PSUM") as psumBT,
                tc.tile_pool(name="psumC", bufs=2, space="PSUM") as psumC,
            ):
                for sq in range(NS):
                    qblk = bass.ts(sq, P)
                    # B: logits stripe -> exp -> row sums -> transpose
                    alpha = workBC.tile([P, S], BF16, tag="alpha")
                    dpart = stats_pool.tile([P, S // NC], F32, tag="dpart")
                    for c in range(S // NC):
                        lp = psumB.tile(
                            [P, NC], F32, tag=f"lp{c % 2}", name=f"lp{c}"
                        )
                        for g in range(NH // 2):
                            nc.tensor.matmul(
                                lp,
                                qT[:, 2 * g : 2 * g + 2, qblk],
                                kT[:, 2 * g : 2 * g + 2, c * NC : (c + 1) * NC],
                                perf_mode=mybir.MatmulPerfMode.DoubleRow,
                                start=(g == 0),
                                stop=(g == NH // 2 - 1),
                            )
                        nc.scalar.activation(
                            out=alpha[:, c * NC : (c + 1) * NC],
                            in_=lp,
                            func=AF.Exp,
                            scale=1.0 / H,
                            accum_out=dpart[:, c : c + 1],
                        )
                    den = stats_pool.tile([P, 1], F32, tag="den")
                    nc.vector.reduce_sum(
                        out=den, in_=dpart, axis=mybir.AxisListType.X
                    )
                    nc.vector.reciprocal(out=recips[:, sq : sq + 1], in_=den)

                    # Transposed alpha stripe [Sk, this 128-q-block].
                    aT = workBC.tile([P, NS, P], FP8, tag="aT_st")
                    for g in range(NS // 4):
                        tpb = psumBT.tile([P, 4, P], BF16, tag="tpb", name=f"tpb{g}")
                        for j in range(4):
                            nc.tensor.transpose(
                                tpb[:, j, :],
                                alpha[:, (4 * g + j) * P : (4 * g + j + 1) * P],
                                identb,
                            )
                        # Delta softmax: exp(l)-1 applied during the cast to
                        # fp8 (values ~±0.2 quantize ~20x better than ~1.0);
                        # the exact colsum(Y) is added back in phase C.
                        nc.vector.tensor_scalar_add(
                            aT[:, 4 * g : 4 * g + 4, :], tpb, -1.0
                        )

                    # C: U stripe = alphaT^T @ Y, scaled by 1/denom on the way
                    up = [
                        psumC.tile([P, NC], F32, tag=f"up{c}", name=f"up{c}")
                        for c in range(F // NC)
                    ]
                    for k2 in range(NS // 2):
                        for c in range(F // NC):
                            nc.tensor.matmul(
                                up[c],
                                aT[:, 2 * k2 : 2 * k2 + 2, :],
                                y_sb[:, 2 * k2 : 2 * k2 + 2, c * NC : (c + 1) * NC],
                                perf_mode=mybir.MatmulPerfMode.DoubleRow,
                                start=(k2 == 0),
                                stop=False,
                            )
                    for c in range(F // NC):
                        # += colsum(Y): rank-1 f32r matmul (ones^T x colsum)
                        nc.tensor.matmul(
                            up[c],
                            ones1,
                            crow[0:1, c * NC : (c + 1) * NC],
                            start=False,
                            stop=True,
                        )
                    o_st = workBC.tile([P, F], F32, tag="o_st")
                    for c in range(F // NC):
                        nc.scalar.activation(
                            out=o_st[:, c * NC : (c + 1) * NC],
                            in_=up[c],
                            func=AF.Copy,
                            scale=recips[:, sq : sq + 1],
                        )
                    nc.sync.dma_start(out=out[sq * P : (sq + 1) * P, :], in_=o_st)

    nc.finalize()
    return nc


_NC_CACHE: dict = {}


def kernel(X, Y, K, Q, g1, b1, g2, b2, _trace=False, _trace_kwargs=None):
    B = X.shape[0]
    assert X.shape == (B, S, F) and Y.shape == (B, S, F)
    bf = ml_dtypes.bfloat16
    f8 = ml_dtypes.float8_e4m3

    affine1 = not (np.all(g1 == 1.0) and np.all(b1 == 0.0))
    affine2 = not (np.all(g2 == 1.0) and np.all(b2 == 0.0))

    key = (affine1, affine2)
    if key not in _NC_CACHE:
        _NC_CACHE[key] = _build_nc(affine1, affine2)
    nc = _NC_CACHE[key]

    kw_b = np.ascontiguousarray(K).astype(f8)
    qw_b = np.ascontiguousarray(Q).astype(f8)
    in_maps = []
    for b in range(B):
        m = {
            "XT": np.ascontiguousarray(X[b].T).astype(f8),
            "YT": np.ascontiguousarray(Y[b].T).astype(f8),
            "Y8": np.ascontiguousarray(Y[b]).astype(f8),
            "CS": Y[b].astype(np.float32).sum(0, keepdims=True),
            "ONES": np.ones((1, P), np.float32),
            "Kw": kw_b,
            "Qw": qw_b,
        }
        if affine1:
            m["g1r"] = g1.astype(bf).reshape(1, H)
            m["b1r"] = b1.astype(bf).reshape(1, H)
        if affine2:
            m["g2r"] = g2.astype(bf).reshape(1, H)
            m["b2r"] = b2.astype(bf).reshape(1, H)
        in_maps.append(m)

    res = run_bass_kernel_spmd(
        nc,
        in_maps,
        core_ids=list(range(B)),
        trace=_trace,
        **(_trace_kwargs or {}),
    )
    kernel.last_result = res
    return np.stack([r["out"] for r in res.results], axis=0).astype(np.float32)


# revision 21
# speedup vs baseline: 1.0236x; 1.0236x over previous
"""Trainium2 Bass kernel for batched attention with LayerNorm'd projections.

Reference computation (per batch element b):
    keys    = LN(Y[b] @ K) * g1 + b1          [S, H]
    queries = LN(X[b] @ Q) * g2 + b2          [S, H]
    alpha   = softmax(queries @ keys.T / H)   [S, S]
    out[b]  = alpha @ Y[b]                    [S, F]

Shapes: B=8, S=2048, F=H=1024. Data-parallel: one batch element per
NeuronCore, 8 cores, no collectives.

Device pipeline per core:
  A: q/k projections in natural [S,H] layout (host supplies X^T, Y^T in
     fp8-e4m3 so the full operand set stays SBUF-resident; the contraction
     dim F lands on partitions), f32 PSUM accumulate, LayerNorm along the
     free dim, then PE-transpose 128x128 bf16 blocks into H-major
     queriesT/keysT.
  B: logits stripes [128, 2048] = queriesT_block^T @ keysT (bf16),
     exp(x/1024) fused on the scalar engine with row-sum accumulation
     (softmax denominator comes out of accum_out for free), PE-transpose
     the alpha stripe into Sk-major layout.
  C: U = alphaT^T @ Y (bf16) in natural [S,F] layout; the mandatory
     PSUM->SBUF copy applies the per-row 1/denominator, so no separate
     softmax normalization pass exists anywhere.

Numerics: fp8 projection inputs cost ~4e-3 relative error end-to-end
(validated against the f32 reference offline); logits/softmax/value paths
stay bf16 with f32 accumulation.

DMA instructions can carry at most 2 semaphore waits on this target (and
the XBAR transpose form only 1), so mid-pipeline transposes run on the
TensorEngine (identity matmul -> PSUM -> DVE copy), and all DMA-written
SBUF tiles are written exactly once (full fp8 residency removes streaming
slot reuse in phase A).
"""

import numpy as np
import ml_dtypes

import concourse.bass as bass
import concourse.bacc as bacc
import concourse.tile as tile
from concourse import mybir
from concourse.bass_utils import run_bass_kernel_spmd
from concourse.masks import make_identity

BF16 = mybir.dt.bfloat16
FP8 = mybir.dt.float8e4
F32 = mybir.dt.float32
AF = mybir.ActivationFunctionType

S = 2048  # sequence length per core
SP = 3072  # padded qT/kT row stride (odd multiple of 1KB: avoids SBUF bank conflicts in DoubleRow pair fetch)
F = 1024  # input feature dim
H = 1024  # hidden dim
P = 128  # partitions
NS = S // P  # 16 sequence stripes
NF = F // P  # 8 contraction tiles for projections
NH = H // P  # 8 hidden tiles
NC = 512  # matmul free-dim chunk (one PSUM bank)
EPS = 1e-5


def _build_nc(affine1: bool, affine2: bool) -> bass.Bass:
    nc = bacc.Bacc(None)

    xt = nc.declare_dram_parameter("XT", [F, S], FP8, isOutput=False)[:]
    yt = nc.declare_dram_parameter("YT", [F, S], FP8, isOutput=False)[:]
    y8 = nc.declare_dram_parameter("Y8", [S, F], FP8, isOutput=False)[:]
    cs = nc.declare_dram_parameter("CS", [1, F], mybir.dt.float32r, isOutput=False)[:]
    onesp = nc.declare_dram_parameter("ONES", [1, P], mybir.dt.float32r, isOutput=False)[:]
    kw = nc.declare_dram_parameter("Kw", [F, H], FP8, isOutput=False)[:]
    qw = nc.declare_dram_parameter("Qw", [F, H], FP8, isOutput=False)[:]
    g1 = b1 = g2 = b2 = None
    if affine1:
        g1 = nc.declare_dram_parameter("g1r", [1, H], BF16, isOutput=False)[:]
        b1 = nc.declare_dram_parameter("b1r", [1, H], BF16, isOutput=False)[:]
    if affine2:
        g2 = nc.declare_dram_parameter("g2r", [1, H], BF16, isOutput=False)[:]
        b2 = nc.declare_dram_parameter("b2r", [1, H], BF16, isOutput=False)[:]
    out = nc.declare_dram_parameter("out", [S, F], F32, isOutput=True)[:]

    with tile.TileContext(nc) as tc:
        with (
            tc.tile_pool(name="persist", bufs=1) as persist,
            tc.tile_pool(name="stats", bufs=8) as stats_pool,
        ):
            # Persistent SBUF tensors (whole-kernel lifetime).
            # Per-partition: qT 32k + kT 32k + y_sb 32k + recips ~0.1k = 96k.
            qT = persist.tile([P, NH, SP], FP8, tag="qT")  # queriesT [H, S+pad]
            kT = persist.tile([P, NH, SP], FP8, tag="kT")  # keysT    [H, S+pad]
            recips = persist.tile([P, NS], F32, tag="recips")
            y_sb = persist.tile([P, NS, F], FP8, tag="y_sb")  # Y [Sk, F]
            crow = persist.tile([1, F], mybir.dt.float32r, tag="crow")  # colsum(Y)
            ones1 = persist.tile([1, P], mybir.dt.float32r, tag="ones1")
            eps_sb = persist.tile([P, 1], F32, tag="eps")
            nc.vector.memset(eps_sb, EPS)
            identb = persist.tile([P, P], BF16, tag="identb")
            make_identity(nc, identb)

            def layer_norm_apply(ps, dst, gamma, beta):
                """dst = LN(ps) (*gamma+beta), free dim H, ps in PSUM.

                Work is split across DVE (stats, recip, apply half 0) and
                ACT (sqrt, apply half 1) -- phase A is epilogue-bound, not
                matmul-bound, so engine balance sets the stripe rate.
                """
                # bn_stats free-dim limit is 512.
                st = stats_pool.tile([P, 2, 6], F32, tag="bn")
                for i in range(2):
                    nc.vector.bn_stats(
                        out=st[:, i, :], in_=ps[:, i * NC : (i + 1) * NC]
                    )
                mv = stats_pool.tile([P, 2], F32, tag="mv")
                nc.vector.bn_aggr(out=mv, in_=st)
                rstd = stats_pool.tile([P, 1], F32, tag="rstd")
                nc.scalar.activation(
                    out=rstd, in_=mv[:, 1:2], func=AF.Sqrt, bias=eps_sb
                )
                nc.vector.reciprocal(out=rstd, in_=rstd)
                nbias = stats_pool.tile([P, 1], F32, tag="nbias")
                nc.vector.tensor_scalar(
                    out=nbias,
                    in0=mv[:, 0:1],
                    scalar1=rstd,
                    scalar2=-1.0,
                    op0=mybir.AluOpType.mult,
                    op1=mybir.AluOpType.mult,
                )
                nc.vector.tensor_scalar(
                    out=dst[:, 0:NC],
                    in0=ps[:, 0:NC],
                    scalar1=mv[:, 0:1],
                    scalar2=rstd,
                    op0=mybir.AluOpType.subtract,
                    op1=mybir.AluOpType.mult,
                )
                nc.scalar.activation(
                    out=dst[:, NC : 2 * NC],
                    in_=ps[:, NC : 2 * NC],
                    func=AF.Identity,
                    bias=nbias,
                    scale=rstd,
                )
                if gamma is not None:
                    nc.vector.tensor_mul(dst, dst, gamma)
                if beta is not None:
                    nc.vector.tensor_add(dst, dst, beta)

            # ---- Phase A: projections + LN + transpose to H-major ----
            with (
                tc.tile_pool(name="operands", bufs=1) as operands,
                tc.tile_pool(name="work", bufs=3) as work,
                tc.tile_pool(name="psumA", bufs=1, space="PSUM") as psumA,
                tc.tile_pool(name="psumAT", bufs=2, space="PSUM") as psumAT,
            ):
                # All projection operands SBUF-resident in fp8:
                # xt/yt 16k + q/k 8k each = 48k per partition.
                xt_sb = operands.tile([P, NF, S], FP8, tag="xt_sb")
                yt_sb = operands.tile([P, NF, S], FP8, tag="yt_sb")
                q_sb = operands.tile([P, NF, H], FP8, tag="q_sb")
                k_sb = operands.tile([P, NF, H], FP8, tag="k_sb")
                xt_r = xt.rearrange("(fb p) s -> p fb s", p=P)
                yt_r = yt.rearrange("(fb p) s -> p fb s", p=P)
                qw_r = qw.rearrange("(fb p) h -> p fb h", p=P)
                kw_r = kw.rearrange("(fb p) h -> p fb h", p=P)
                # Per-f-block loads so the first matmuls start after ~400KB,
                # not after the full operand set; k/yt first (k stripes run
                # first below).
                for f in range(NF):
                    nc.sync.dma_start(out=yt_sb[:, f, :], in_=yt_r[:, f, :])
                    nc.sync.dma_start(out=k_sb[:, f, :], in_=kw_r[:, f, :])
                for f in range(NF):
                    nc.sync.dma_start(out=xt_sb[:, f, :], in_=xt_r[:, f, :])
                    nc.sync.dma_start(out=q_sb[:, f, :], in_=qw_r[:, f, :])
                aff_tiles = {}
                for name, flag, ap in (
                    ("g1", affine1, g1),
                    ("b1", affine1, b1),
                    ("g2", affine2, g2),
                    ("b2", affine2, b2),
                ):
                    if flag:
                        t = operands.tile([P, H], BF16, tag=name, name=f"aff_{name}")
                        rep = bass.AP(
                            tensor=ap.tensor, offset=ap.offset, ap=[[0, P], ap.ap[1]]
                        )
                        nc.sync.dma_start(out=t, in_=rep)
                        aff_tiles[name] = t

                DR = mybir.MatmulPerfMode.DoubleRow
                mats = [("k", s) for s in range(NS)] + [("q", s) for s in range(NS)]
                for mi, (which, s) in enumerate(mats):
                    sblk = bass.ts(s, P)
                    lhs_all = xt_sb if which == "q" else yt_sb
                    rhs_all = q_sb if which == "q" else k_sb
                    # 3 rotating PSUM bank sets so the LayerNorm stats chain
                    # of stripe i drains while stripes i+1, i+2 accumulate.
                    pset = mi % 3
                    ps = psumA.tile([P, H], F32, tag=f"p{pset}", name=f"ps_{mi}")
                    for i in range(NF // 2):
                        for c in range(H // NC):
                            nc.tensor.matmul(
                                ps[:, c * NC : (c + 1) * NC],
                                lhs_all[:, 2 * i : 2 * i + 2, sblk],
                                rhs_all[:, 2 * i : 2 * i + 2, c * NC : (c + 1) * NC],
                                perf_mode=DR,
                                start=(i == 0),
                                stop=(i == NF // 2 - 1),
                            )
                    nat = work.tile([P, H], BF16, tag=f"{which}_nat")
                    if which == "q":
                        layer_norm_apply(
                            ps, nat, aff_tiles.get("g2"), aff_tiles.get("b2")
                        )
                    else:
                        layer_norm_apply(
                            ps, nat, aff_tiles.get("g1"), aff_tiles.get("b1")
                        )
                    dstT = qT if which == "q" else kT
                    for g in range(NH // 4):
                        tp = psumAT.tile(
                            [P, 4, P], BF16, tag="tpA", name=f"tp_{which}{g}"
                        )
                        for j in range(4):
                            nc.tensor.transpose(
                                tp[:, j, :],
                                nat[:, (4 * g + j) * P : (4 * g + j + 1) * P],
                                identb,
                            )
                        if g == 0:
                            nc.vector.tensor_copy(
                                dstT[:, 4 * g : 4 * g + 4, sblk], tp
                            )
                        else:
                            nc.scalar.copy(
                                dstT[:, 4 * g : 4 * g + 4, sblk], tp
                            )
                # Y values for phase C: issued after the phase-A loads in
                # trace order so they don't delay the first matmuls.
                nc.sync.dma_start(
                    out=y_sb, in_=y8.rearrange("(sb p) f -> p sb f", p=P)
                )
                nc.sync.dma_start(out=crow, in_=cs)
                nc.sync.dma_start(out=ones1, in_=onesp)

            # ---- Phases B and C (interleaved per stripe) ----
            with (
                tc.tile_pool(name="workBC", bufs=3) as workBC,
                tc.tile_pool(name="psumB", bufs=1, space="PSUM") as psumB,
                tc.tile_pool(name="psumBT", bufs=2, space="PSUM") as psumBT,
                tc.tile_pool(name="psumC", bufs=2, space="PSUM") as psumC,
            ):
                for sq in range(NS):
                    qblk = bass.ts(sq, P)
                    # B: logits stripe -> exp -> row sums -> transpose
                    alpha = workBC.tile([P, S], BF16, tag="alpha")
                    dpart = stats_pool.tile([P, S // NC], F32, tag="dpart")
                    for c in range(S // NC):
                        lp = psumB.tile(
                            [P, NC], F32, tag=f"lp{c % 2}", name=f"lp{c}"
                        )
                        for g in range(NH // 2):
                            nc.tensor.matmul(
                                lp,
                                qT[:, 2 * g : 2 * g + 2, qblk],
                                kT[:, 2 * g : 2 * g + 2, c * NC : (c + 1) * NC],
                                perf_mode=mybir.MatmulPerfMode.DoubleRow,
                                start=(g == 0),
                                stop=(g == NH // 2 - 1),
                            )
                        nc.scalar.activation(
                            out=alpha[:, c * NC : (c + 1) * NC],
                            in_=lp,
                            func=AF.Exp,
                            scale=1.0 / H,
                            accum_out=dpart[:, c : c + 1],
                        )
                    den = stats_pool.tile([P, 1], F32, tag="den")
                    nc.vector.reduce_sum(
                        out=den, in_=dpart, axis=mybir.AxisListType.X
                    )
                    nc.vector.reciprocal(out=recips[:, sq : sq + 1], in_=den)

                    # Transposed alpha stripe [Sk, this 128-q-block].
                    aT = workBC.tile([P, NS, P], FP8, tag="aT_st")
                    for g in range(NS // 4):
                        tpb = psumBT.tile([P, 4, P], BF16, tag="tpb", name=f"tpb{g}")
                        for j in range(4):
                            nc.tensor.transpose(
                                tpb[:, j, :],
                                alpha[:, (4 * g + j) * P : (4 * g + j + 1) * P],
                                identb,
                            )
                        # Delta softmax: exp(l)-1 applied during the cast to
                        # fp8 (values ~±0.2 quantize ~20x better than ~1.0);
                        # the exact colsum(Y) is added back in phase C.
                        nc.vector.tensor_scalar_add(
                            aT[:, 4 * g : 4 * g + 4, :], tpb, -1.0
                        )

                    # C: U stripe = alphaT^T @ Y, scaled by 1/denom on the way
                    up = [
                        psumC.tile([P, NC], F32, tag=f"up{c}", name=f"up{c}")
                        for c in range(F // NC)
                    ]
                    for k2 in range(NS // 2):
                        for c in range(F // NC):
                            nc.tensor.matmul(
                                up[c],
                                aT[:, 2 * k2 : 2 * k2 + 2, :],
                                y_sb[:, 2 * k2 : 2 * k2 + 2, c * NC : (c + 1) * NC],
                                perf_mode=mybir.MatmulPerfMode.DoubleRow,
                                start=(k2 == 0),
                                stop=False,
                            )
                    for c in range(F // NC):
                        # += colsum(Y): rank-1 f32r matmul (ones^T x colsum)
                        nc.tensor.matmul(
                            up[c],
                            ones1,
                            crow[0:1, c * NC : (c + 1) * NC],
                            start=False,
                            stop=True,
                        )
                    o_st = workBC.tile([P, F], F32, tag="o_st")
                    for c in range(F // NC):
                        nc.scalar.activation(
                            out=o_st[:, c * NC : (c + 1) * NC],
                            in_=up[c],
                            func=AF.Copy,
                            scale=recips[:, sq : sq + 1],
                        )
                    nc.sync.dma_start(out=out[sq * P : (sq + 1) * P, :], in_=o_st)

    nc.finalize()
    return nc


_NC_CACHE: dict = {}


def kernel(X, Y, K, Q, g1, b1, g2, b2, _trace=False, _trace_kwargs=None):
    B = X.shape[0]
    assert X.shape == (B, S, F) and Y.shape == (B, S, F)
    bf = ml_dtypes.bfloat16
    f8 = ml_dtypes.float8_e4m3

    affine1 = not (np.all(g1 == 1.0) and np.all(b1 == 0.0))
    affine2 = not (np.all(g2 == 1.0) and np.all(b2 == 0.0))

    key = (affine1, affine2)
    if key not in _NC_CACHE:
        _NC_CACHE[key] = _build_nc(affine1, affine2)
    nc = _NC_CACHE[key]

    kw_b = np.ascontiguousarray(K).astype(f8)
    qw_b = np.ascontiguousarray(Q).astype(f8)
    in_maps = []
    for b in range(B):
        m = {
            "XT": np.ascontiguousarray(X[b].T).astype(f8),
            "YT": np.ascontiguousarray(Y[b].T).astype(f8),
            "Y8": np.ascontiguousarray(Y[b]).astype(f8),
            "CS": Y[b].astype(np.float32).sum(0, keepdims=True),
            "ONES": np.ones((1, P), np.float32),
            "Kw": kw_b,
            "Qw": qw_b,
        }
        if affine1:
            m["g1r"] = g1.astype(bf).reshape(1, H)
            m["b1r"] = b1.astype(bf).reshape(1, H)
        if affine2:
            m["g2r"] = g2.astype(bf).reshape(1, H)
            m["b2r"] = b2.astype(bf).reshape(1, H)
        in_maps.append(m)

    res = run_bass_kernel_spmd(
        nc,
        in_maps,
        core_ids=list(range(B)),
        trace=_trace,
        **(_trace_kwargs or {}),
    )
    kernel.last_result = res
    return np.stack([r["out"] for r in res.results], axis=0).astype(np.float32)
